# revision 14
# baseline (speedup 1.0000x reference)
"""Trainium2 Bass kernel for nn_ChildHAggregation (gnn_message_passing).

Computation per sample (see docstring math below):
  x = [hl, hr]                                        (1024)
  q_t = (h_t @ qU + qU_b) * su_q + sb_q   t in {l,r}  su/sb from xh
  k_t = (h_t @ kU + kU_b) * su_k + sb_k
  2x2 softmax attention over the two tokens -> per-sample probs p_ij
  x += scores @ [hl, hr]; layernorm(ddof=1) with alpha/beta
  out = (x @ hU + hU_b) * (xh @ hWu + hWu_b) + (xh @ hWb + hWb_b)
      + (xw @ lU + lU_b) * (xh @ lWu + lWu_b) + (xh @ lWb + lWb_b)

Strategy (pure data-parallel over 8 cores, batch 32768 -> 4096/core):
  - batch-major tiles [128 samples x features]; feature-contraction
    matmuls use PE-transposed input tiles as the stationary operand so
    outputs land batch-major in PSUM.
  - 2-token softmax == sigmoid of score differences -> per-sample
    scalars; attention + layernorm are folded into per-sample scalar
    algebra applied AFTER the matmuls:
      x @ (alpha*hU) = a0*M1 + b0*M2 + a1*M3 + b1*M4  (M* = h_t @ hU'half)
      norm fixup: inv * (hu_x - mean * colsum(alpha*hU)), beta@hU folded
      into a bias row.
  - all "U-bias * hyper" terms folded into precomputed weights:
      qWb' = qWb + qWu*diag(qU_b) etc., so q = (h@qU)*su + (xh@qWb'+qb').
    hidden/leaf additive paths combined: WC = hWb' + lWb' (one matmul).
"""

import os
from contextlib import ExitStack

import numpy as np

import concourse.bacc as bacc
import concourse.bass as bass
import concourse.mybir as mybir
import concourse.tile as tile
from concourse.bass_utils import run_bass_kernel_spmd
from concourse.masks import make_identity

N_CORES = 8
B_FULL = 32768
HALF = 512
DIM = 1024
P = 128
EPS = 1e-6
INV_SQRT_HALF = 1.0 / float(np.sqrt(np.float32(HALF)))

f32 = mybir.dt.float32
f32r = mybir.dt.float32r

AX = mybir.AxisListType
ALU = mybir.AluOpType
ACTF = mybir.ActivationFunctionType

W512 = ["qU", "kU", "qWu", "qWb", "kWu", "kWb", "hWu", "hWb", "lWu", "lWb"]


def _mm(ap, mm_dt):
    """Bitcast an fp32 AP to the matmul dtype (f32 or f32r; same bits)."""
    if mm_dt == f32:
        return ap
    return ap.bitcast(mm_dt)


def build_nc(b_loc, mm_dt=f32r, stage=99):
    """Build the per-core Bass program for a local batch of b_loc rows.

    stage truncates the program for hang-bisection:
      1=setup only, 2=+transposes, 3=+phaseA mm/evict, 4=+qk/dots/probs,
      5=+stats algebra, 99=full
    """
    n_tiles = b_loc // P
    assert n_tiles * P == b_loc

    nc = bacc.Bacc("TRN2", target_bir_lowering=False, debug=False,
                   num_devices=1)

    # ---- DRAM I/O (names match setup_inputs) ----
    d = {}
    d["hl"] = nc.dram_tensor("hl", [b_loc, HALF], f32, kind="ExternalInput").ap()
    d["hr"] = nc.dram_tensor("hr", [b_loc, HALF], f32, kind="ExternalInput").ap()
    d["xw"] = nc.dram_tensor("xw", [b_loc, DIM], f32, kind="ExternalInput").ap()
    d["xh"] = nc.dram_tensor("xh", [b_loc, HALF], f32, kind="ExternalInput").ap()
    for w in W512:
        d[w + "_w"] = nc.dram_tensor(w + "_w", [HALF, HALF], f32,
                                     kind="ExternalInput").ap()
        d[w + "_b"] = nc.dram_tensor(w + "_b", [HALF], f32,
                                     kind="ExternalInput").ap()
    for w in ["hU", "lU"]:
        d[w + "_w"] = nc.dram_tensor(w + "_w", [DIM, HALF], f32,
                                     kind="ExternalInput").ap()
        d[w + "_b"] = nc.dram_tensor(w + "_b", [HALF], f32,
                                     kind="ExternalInput").ap()
    d["alpha"] = nc.dram_tensor("alpha", [DIM], f32, kind="ExternalInput").ap()
    d["beta"] = nc.dram_tensor("beta", [DIM], f32, kind="ExternalInput").ap()
    out_d = nc.dram_tensor("out", [b_loc, HALF], f32, kind="ExternalOutput").ap()

    with tile.TileContext(nc) as tc, ExitStack() as ctx:
        # ================= persistent pools =================
        wts = ctx.enter_context(tc.tile_pool(name="wts", bufs=1))
        biasp = ctx.enter_context(tc.tile_pool(name="biasp", bufs=1))

        # final weight tiles (written exactly once, in the matmul dtype, so
        # the fp32r BIR-verifier sees rounded producers)
        wsb = {}
        for w in ["qU", "kU", "qWu", "kWu", "hWu", "lWu", "qWb", "kWb", "hWb"]:
            wsb[w] = wts.tile([P, 4, HALF], f32, name=f"w_{w}")
        for w in ["hU", "lU"]:
            wsb[w] = wts.tile([P, 8, HALF], f32, name=f"w_{w}")
        alpha_sb = wts.tile([P, 8], f32)
        nc.sync.dma_start(alpha_sb, d["alpha"].rearrange("(c p) -> p c", p=P))
        beta_sb = wts.tile([P, 8], f32)
        nc.sync.dma_start(beta_sb, d["beta"].rearrange("(c p) -> p c", p=P))
        ident = wts.tile([P, P], f32)
        make_identity(nc, ident)

        # persistent broadcast [P, 512] bias tiles
        bc = {}
        for nm in ["qWu_b", "kWu_b", "hWu_b", "lWu_b", "qb", "kb", "cb", "cs"]:
            bc[nm] = biasp.tile([P, HALF], f32, name=f"bc_{nm}")

        # ---------------- one-time setup ----------------
        with tc.tile_pool(name="setup", bufs=1) as sp, \
                tc.tile_pool(name="psum_setup", bufs=2, space="PSUM") as psum_setup:

            def bias_row(nm):
                r = sp.tile([1, HALF], f32, tag="row", bufs=2, name=f"row_{nm}")
                nc.sync.dma_start(r, d[nm][None, :])
                return r

            def bcast(dst, row_ap):
                nc.gpsimd.partition_broadcast(dst, row_ap)

            def tmp_bc(nm, row_ap):
                t = sp.tile([P, HALF], f32, tag="tbc", bufs=5, name=f"tbc_{nm}")
                bcast(t, row_ap)
                return t

            # su-evict biases straight to persistent broadcasts
            for nm in ["qWu_b", "kWu_b", "hWu_b", "lWu_b"]:
                bcast(bc[nm], bias_row(nm))

            # temp broadcasts used by folds / combined rows
            qUb_bc = tmp_bc("qU_b", bias_row("qU_b"))
            kUb_bc = tmp_bc("kU_b", bias_row("kU_b"))
            lUb_bc = tmp_bc("lU_b", bias_row("lU_b"))

            # combined bias rows (computed on broadcast tiles):
            # qb' = qWb_b + qU_b*qWu_b ; kb' likewise
            ftmp = sp.tile([P, HALF], f32, tag="ftmp")
            bcast(bc["qb"], bias_row("qWb_b"))
            nc.vector.tensor_mul(ftmp, qUb_bc, bc["qWu_b"])
            nc.vector.tensor_add(bc["qb"], bc["qb"], ftmp)
            bcast(bc["kb"], bias_row("kWb_b"))
            nc.vector.tensor_mul(ftmp, kUb_bc, bc["kWu_b"])
            nc.vector.tensor_add(bc["kb"], bc["kb"], ftmp)

            # weights: DMA into rotating temps, finals written once (rounded)
            def wtemp(w, nch):
                t = sp.tile([P, nch, HALF], f32, tag=f"wtmp{nch}",
                            bufs=(3 if nch == 4 else 1), name=f"wtmp_{w}")
                nc.sync.dma_start(t, d[w + "_w"].rearrange("(c p) o -> p c o", p=P))
                return t

            for w in ["qU", "kU", "qWu", "kWu", "hWu", "lWu"]:
                t = wtemp(w, 4)
                for c in range(4):
                    nc.vector.tensor_copy(_mm(wsb[w][:, c, :], mm_dt), t[:, c, :])

            # hU: bh/cs matmuls on raw hU, then alpha-scale into final
            hU_tmp = wtemp("hU", 8)
            bh_ps = psum_setup.tile([1, HALF], f32)
            cs_ps = psum_setup.tile([1, HALF], f32)
            for c in range(8):
                nc.tensor.matmul(bh_ps, beta_sb[:, c:c + 1], hU_tmp[:, c, :],
                                 start=(c == 0), stop=(c == 7))
            for c in range(8):
                nc.tensor.matmul(cs_ps, alpha_sb[:, c:c + 1], hU_tmp[:, c, :],
                                 start=(c == 0), stop=(c == 7))
            bh_row = sp.tile([1, HALF], f32, tag="row", bufs=2)
            nc.vector.tensor_add(bh_row, bh_ps, bias_row("hU_b"))
            cs_row = sp.tile([1, HALF], f32, tag="row", bufs=2)
            nc.vector.tensor_copy(cs_row, cs_ps)
            bcast(bc["cs"], cs_row)
            bh_bc = tmp_bc("bh", bh_row)
            for c in range(8):
                nc.scalar.activation(_mm(wsb["hU"][:, c, :], mm_dt),
                                     hU_tmp[:, c, :],
                                     ACTF.Copy, scale=alpha_sb[:, c:c + 1])

            lU_tmp = wtemp("lU", 8)
            for c in range(8):
                nc.vector.tensor_copy(_mm(wsb["lU"][:, c, :], mm_dt),
                                      lU_tmp[:, c, :])

            # cb = (hWb_b + bh*hWu_b) + (lWb_b + lU_b*lWu_b)
            bcast(bc["cb"], bias_row("hWb_b"))
            nc.vector.tensor_mul(ftmp, bh_bc, bc["hWu_b"])
            nc.vector.tensor_add(bc["cb"], bc["cb"], ftmp)
            lWbb_bc = tmp_bc("lWb_b", bias_row("lWb_b"))
            nc.vector.tensor_add(bc["cb"], bc["cb"], lWbb_bc)
            nc.vector.tensor_mul(ftmp, lUb_bc, bc["lWu_b"])
            nc.vector.tensor_add(bc["cb"], bc["cb"], ftmp)

            # folded weight matrices:
            # qWb' = qWb + qWu*diag(qU_b) ; kWb' = kWb + kWu*diag(kU_b)
            # WC   = hWb + hWu*diag(bh) + lWb + lWu*diag(lU_b)
            qWb_tmp = wtemp("qWb", 4)
            for c in range(4):
                nc.vector.tensor_mul(ftmp, wsb["qWu"][:, c, :], qUb_bc)
                nc.vector.tensor_add(_mm(wsb["qWb"][:, c, :], mm_dt),
                                     qWb_tmp[:, c, :], ftmp)
            kWb_tmp = wtemp("kWb", 4)
            for c in range(4):
                nc.vector.tensor_mul(ftmp, wsb["kWu"][:, c, :], kUb_bc)
                nc.vector.tensor_add(_mm(wsb["kWb"][:, c, :], mm_dt),
                                     kWb_tmp[:, c, :], ftmp)
            hWb_tmp = wtemp("hWb", 4)
            lWb_tmp = wtemp("lWb", 4)
            for c in range(4):
                nc.vector.tensor_mul(ftmp, wsb["hWu"][:, c, :], bh_bc)
                nc.vector.tensor_add(hWb_tmp[:, c, :], hWb_tmp[:, c, :], ftmp)
                nc.vector.tensor_add(hWb_tmp[:, c, :], hWb_tmp[:, c, :],
                                     lWb_tmp[:, c, :])
                nc.vector.tensor_mul(ftmp, wsb["lWu"][:, c, :], lUb_bc)
                nc.vector.tensor_add(_mm(wsb["hWb"][:, c, :], mm_dt),
                                     hWb_tmp[:, c, :], ftmp)

        # ================= main loop pools =================
        inp = ctx.enter_context(tc.tile_pool(name="inp", bufs=2))
        tsp = ctx.enter_context(tc.tile_pool(name="tsp", bufs=2))
        pha = ctx.enter_context(tc.tile_pool(name="pha", bufs=1))
        scr = ctx.enter_context(tc.tile_pool(name="scr", bufs=4))
        tinyp = ctx.enter_context(tc.tile_pool(name="tinyp", bufs=2))
        phd = ctx.enter_context(tc.tile_pool(name="phd", bufs=1))
        outp = ctx.enter_context(tc.tile_pool(name="outp", bufs=2))
        tp_ps = ctx.enter_context(tc.tile_pool(name="tp_ps", bufs=2, space="PSUM"))
        mm_ps = ctx.enter_context(tc.tile_pool(name="mm_ps", bufs=6, space="PSUM"))

        for i in range(n_tiles):
            rs = bass.ts(i, P)
            # ---- loads ----
            hl_t = inp.tile([P, HALF], f32, tag="hl")
            nc.sync.dma_start(hl_t, d["hl"][rs, :])
            hr_t = inp.tile([P, HALF], f32, tag="hr")
            nc.sync.dma_start(hr_t, d["hr"][rs, :])
            xh_t = inp.tile([P, HALF], f32, tag="xh")
            nc.sync.dma_start(xh_t, d["xh"][rs, :])
            xw_t = inp.tile([P, DIM], f32, tag="xw")
            nc.sync.dma_start(xw_t, d["xw"][rs, :])


            def stage_out(src_ap):
                ot = outp.tile([P, HALF], f32, tag="out_t", name=f"out_stage_{i}")
                nc.vector.tensor_copy(ot, src_ap)
                nc.sync.dma_start(out_d[rs, :], ot)

            if stage == 1:
                stage_out(hl_t)
                continue

            # ---- PE transposes (feature-major stationaries) ----
            def transpose_to(src, ncols, tg):
                sb = tsp.tile([P, ncols * P], f32, tag=tg, name=f"T_{tg}_{i}")
                for g in range(0, ncols, 4):
                    ps = tp_ps.tile([P, 4 * P], f32, tag="tp",
                                    name=f"tps_{tg}_{g}_{i}")
                    gw = min(4, ncols - g)
                    for c in range(gw):
                        nc.tensor.transpose(
                            ps[:, c * P:(c + 1) * P],
                            src[:, (g + c) * P:(g + c + 1) * P],
                            ident)
                    # evict writes the matmul dtype (rounds for fp32r)
                    nc.scalar.copy(_mm(sb[:, g * P:(g + gw) * P], mm_dt),
                                   ps[:, :gw * P])
                return sb

            hlT = transpose_to(hl_t, 4, "ThL")
            hrT = transpose_to(hr_t, 4, "ThR")
            xhT = transpose_to(xh_t, 4, "TxH")
            xwT = transpose_to(xw_t, 8, "TxW")

            if stage == 2:
                stage_out(hlT)
                continue

            # ---- row stats of hl / hr (for fused layernorm algebra) ----
            skip_stats = (stage == 22)
            smask = int(os.environ.get("KERNEL_STATS_MASK", "7"))
            if not skip_stats:
                sl = tinyp.tile([P, 1], f32, tag="sl")
                sr = tinyp.tile([P, 1], f32, tag="sr")
                ql = tinyp.tile([P, 1], f32, tag="ql")
                qr = tinyp.tile([P, 1], f32, tag="qr")
                cr2 = tinyp.tile([P, 1], f32, tag="cr2")
                if smask & 1:
                    nc.vector.reduce_sum(sl, hl_t, axis=AX.X)
                    nc.vector.reduce_sum(sr, hr_t, axis=AX.X)
                if smask & 2:
                    s1 = scr.tile([P, HALF], f32, tag="scr", name=f"scr_ql_{i}")
                    nc.scalar.activation(s1, hl_t, ACTF.Square, accum_out=ql)
                    s2 = scr.tile([P, HALF], f32, tag="scr", name=f"scr_qr_{i}")
                    nc.scalar.activation(s2, hr_t, ACTF.Square, accum_out=qr)
                if smask & 4:
                    s3 = scr.tile([P, HALF], f32, tag="scr", name=f"scr_cr_{i}")
                    nc.vector.scalar_tensor_tensor(
                        s3, hl_t, 0.0, hr_t, ALU.bypass, ALU.mult,
                        accum_out=cr2)
            if stage == 21:
                stage_out(hlT)
                continue

            # ---- phase A matmuls ----
            def unit(tag):
                return mm_ps.tile([P, HALF], f32, tag="mm", name=f"ps_{tag}_{i}")

            SUq, SBq, SUk, SBk = unit("SUq"), unit("SBq"), unit("SUk"), unit("SBk")
            for c in range(4):
                lhs = _mm(xhT[:, bass.ts(c, P)], mm_dt)
                st, sp_ = (c == 0), (c == 3)
                nc.tensor.matmul(SUq, lhs, _mm(wsb["qWu"][:, c, :], mm_dt), start=st, stop=sp_)
                nc.tensor.matmul(SBq, lhs, _mm(wsb["qWb"][:, c, :], mm_dt), start=st, stop=sp_)
                nc.tensor.matmul(SUk, lhs, _mm(wsb["kWu"][:, c, :], mm_dt), start=st, stop=sp_)
                nc.tensor.matmul(SBk, lhs, _mm(wsb["kWb"][:, c, :], mm_dt), start=st, stop=sp_)
            A_l, C_l = unit("A_l"), unit("C_l")
            for c in range(4):
                lhs = _mm(hlT[:, bass.ts(c, P)], mm_dt)
                st, sp_ = (c == 0), (c == 3)
                nc.tensor.matmul(A_l, lhs, _mm(wsb["qU"][:, c, :], mm_dt), start=st, stop=sp_)
                nc.tensor.matmul(C_l, lhs, _mm(wsb["kU"][:, c, :], mm_dt), start=st, stop=sp_)
            A_r, C_r = unit("A_r"), unit("C_r")
            for c in range(4):
                lhs = _mm(hrT[:, bass.ts(c, P)], mm_dt)
                st, sp_ = (c == 0), (c == 3)
                nc.tensor.matmul(A_r, lhs, _mm(wsb["qU"][:, c, :], mm_dt), start=st, stop=sp_)
                nc.tensor.matmul(C_r, lhs, _mm(wsb["kU"][:, c, :], mm_dt), start=st, stop=sp_)

            # ---- phase A elementwise ----
            su = pha.tile([P, HALF], f32, tag="su")
            nc.vector.tensor_add(su, SUq, bc["qWu_b"])
            sbq = pha.tile([P, HALF], f32, tag="sbq")
            nc.vector.tensor_add(sbq, SBq, bc["qb"])
            tu = pha.tile([P, HALF], f32, tag="tu")
            nc.vector.tensor_add(tu, SUk, bc["kWu_b"])
            sbk = pha.tile([P, HALF], f32, tag="sbk")
            nc.vector.tensor_add(sbk, SBk, bc["kb"])

            if stage == 3:
                stage_out(su)
                continue

            q_l = pha.tile([P, HALF], f32, tag="q_l")
            nc.vector.tensor_mul(q_l, A_l, su)
            nc.gpsimd.tensor_add(q_l, q_l, sbq)
            q_r = pha.tile([P, HALF], f32, tag="q_r")
            nc.vector.tensor_mul(q_r, A_r, su)
            nc.gpsimd.tensor_add(q_r, q_r, sbq)
            k_l = pha.tile([P, HALF], f32, tag="k_l")
            nc.vector.tensor_mul(k_l, C_l, tu)
            nc.gpsimd.tensor_add(k_l, k_l, sbk)
            k_r = pha.tile([P, HALF], f32, tag="k_r")
            nc.vector.tensor_mul(k_r, C_r, tu)
            nc.gpsimd.tensor_add(k_r, k_r, sbk)

            # ---- dots -> scores (cols: ll, lr, rl, rr) ----
            stats = tinyp.tile([P, 4], f32, tag="stats")
            for j, (qq, kk) in enumerate([(q_l, k_l), (q_l, k_r),
                                          (q_r, k_l), (q_r, k_r)]):
                sd = scr.tile([P, HALF], f32, tag="scr", name=f"scr_dot{j}_{i}")
                nc.vector.scalar_tensor_tensor(
                    sd, qq, 0.0, kk, ALU.bypass, ALU.mult,
                    accum_out=stats[:, j:j + 1])

            # ---- 2-way softmax via sigmoid ----
            diffs = tinyp.tile([P, 2], f32, tag="diffs")
            nc.vector.tensor_sub(diffs, stats[:, 0:4:2], stats[:, 1:4:2])
            probs = tinyp.tile([P, 2], f32, tag="probs")
            nc.scalar.activation(probs, diffs, ACTF.Sigmoid, scale=INV_SQRT_HALF)
            a0 = tinyp.tile([P, 1], f32, tag="a0")
            nc.scalar.activation(a0, probs[:, 0:1], ACTF.Copy, bias=1.0)
            b0 = tinyp.tile([P, 1], f32, tag="b0")
            nc.scalar.activation(b0, probs[:, 0:1], ACTF.Copy, scale=-1.0, bias=1.0)
            a1 = probs[:, 1:2]
            b1 = tinyp.tile([P, 1], f32, tag="b1")
            nc.scalar.activation(b1, probs[:, 1:2], ACTF.Copy, scale=-1.0, bias=2.0)

            if stage == 4:
                stage_out(q_l)
                continue

            # ---- layernorm stats from folded algebra ----
            e0 = tinyp.tile([P, 1], f32, tag="e0")
            nc.vector.tensor_add(e0, a0, a1)
            e1 = tinyp.tile([P, 1], f32, tag="e1")
            nc.vector.tensor_add(e1, b0, b1)
            sumx = tinyp.tile([P, 1], f32, tag="sumx")
            nc.vector.tensor_mul(sumx, sl, e0)
            nc.vector.scalar_tensor_tensor(sumx, sr, e1, sumx, ALU.mult, ALU.add)
            f0 = tinyp.tile([P, 1], f32, tag="f0")
            nc.vector.tensor_mul(f0, a0, a0)
            nc.vector.scalar_tensor_tensor(f0, a1, a1, f0, ALU.mult, ALU.add)
            f1 = tinyp.tile([P, 1], f32, tag="f1")
            nc.vector.tensor_mul(f1, b0, b0)
            nc.vector.scalar_tensor_tensor(f1, b1, b1, f1, ALU.mult, ALU.add)
            f2 = tinyp.tile([P, 1], f32, tag="f2")
            nc.vector.tensor_mul(f2, a0, b0)
            nc.vector.scalar_tensor_tensor(f2, a1, b1, f2, ALU.mult, ALU.add)
            nc.scalar.activation(f2, f2, ACTF.Copy, scale=2.0)
            ssq = tinyp.tile([P, 1], f32, tag="ssq")
            nc.vector.tensor_mul(ssq, ql, f0)
            nc.vector.scalar_tensor_tensor(ssq, qr, f1, ssq, ALU.mult, ALU.add)
            nc.vector.scalar_tensor_tensor(ssq, cr2, f2, ssq, ALU.mult, ALU.add)
            mean = tinyp.tile([P, 1], f32, tag="mean")
            nc.scalar.activation(mean, sumx, ACTF.Copy, scale=1.0 / DIM)
            m2x = tinyp.tile([P, 1], f32, tag="m2x")
            nc.vector.tensor_mul(m2x, sumx, sumx)
            varn = tinyp.tile([P, 1], f32, tag="varn")
            nc.vector.scalar_tensor_tensor(varn, m2x, -1.0 / DIM, ssq,
                                           ALU.mult, ALU.add)
            stde = tinyp.tile([P, 1], f32, tag="stde")
            nc.scalar.activation(stde, varn, ACTF.Sqrt, scale=1.0 / (DIM - 1))
            nc.scalar.activation(stde, stde, ACTF.Copy, bias=EPS)
            rinv = tinyp.tile([P, 1], f32, tag="rinv")
            nc.vector.reciprocal(rinv, stde)
            nrinv = tinyp.tile([P, 1], f32, tag="nrinv")
            nc.scalar.activation(nrinv, rinv, ACTF.Copy, scale=-1.0)

            if stage == 5:
                stage_out(k_r)
                continue

            # ---- phase D matmuls ----
            M1, M3 = unit("M1"), unit("M3")
            for c in range(4):
                lhs = _mm(hlT[:, bass.ts(c, P)], mm_dt)
                st, sp_ = (c == 0), (c == 3)
                nc.tensor.matmul(M1, lhs, _mm(wsb["hU"][:, c, :], mm_dt), start=st, stop=sp_)
                nc.tensor.matmul(M3, lhs, _mm(wsb["hU"][:, 4 + c, :], mm_dt), start=st, stop=sp_)
            M2, M4 = unit("M2"), unit("M4")
            for c in range(4):
                lhs = _mm(hrT[:, bass.ts(c, P)], mm_dt)
                st, sp_ = (c == 0), (c == 3)
                nc.tensor.matmul(M2, lhs, _mm(wsb["hU"][:, c, :], mm_dt), start=st, stop=sp_)
                nc.tensor.matmul(M4, lhs, _mm(wsb["hU"][:, 4 + c, :], mm_dt), start=st, stop=sp_)
            HSU, LSU, SBC = unit("HSU"), unit("LSU"), unit("SBC")
            for c in range(4):
                lhs = _mm(xhT[:, bass.ts(c, P)], mm_dt)
                st, sp_ = (c == 0), (c == 3)
                nc.tensor.matmul(HSU, lhs, _mm(wsb["hWu"][:, c, :], mm_dt), start=st, stop=sp_)
                nc.tensor.matmul(LSU, lhs, _mm(wsb["lWu"][:, c, :], mm_dt), start=st, stop=sp_)
                nc.tensor.matmul(SBC, lhs, _mm(wsb["hWb"][:, c, :], mm_dt), start=st, stop=sp_)
            LUp = unit("LU")
            for c in range(8):
                nc.tensor.matmul(LUp, _mm(xwT[:, bass.ts(c, P)], mm_dt),
                                 _mm(wsb["lU"][:, c, :], mm_dt),
                                 start=(c == 0), stop=(c == 7))

            # ---- hidden path: hu_x = a0*M1 + b0*M2 + a1*M3 + b1*M4 ----
            t_hu = phd.tile([P, HALF], f32, tag="t_hu")
            nc.scalar.activation(t_hu, M1, ACTF.Copy, scale=a0)
            nc.vector.scalar_tensor_tensor(t_hu, M2, b0, t_hu, ALU.mult, ALU.add)
            nc.vector.scalar_tensor_tensor(t_hu, M3, a1, t_hu, ALU.mult, ALU.add)
            nc.vector.scalar_tensor_tensor(t_hu, M4, b1, t_hu, ALU.mult, ALU.add)
            # t5 = cs*mean - hu_x ; u1 = -inv * t5 = inv*(hu_x - cs*mean)
            t5 = phd.tile([P, HALF], f32, tag="t5")
            nc.vector.scalar_tensor_tensor(t5, bc["cs"], mean, t_hu,
                                           ALU.mult, ALU.subtract)
            nc.scalar.activation(t5, t5, ACTF.Copy, scale=nrinv)

            su_h = phd.tile([P, HALF], f32, tag="su_h")
            nc.vector.tensor_add(su_h, HSU, bc["hWu_b"])
            su_l = phd.tile([P, HALF], f32, tag="su_l")
            nc.vector.tensor_add(su_l, LSU, bc["lWu_b"])
            sbc = phd.tile([P, HALF], f32, tag="sbc")
            nc.vector.tensor_add(sbc, SBC, bc["cb"])

            v1 = phd.tile([P, HALF], f32, tag="v1")
            nc.vector.tensor_mul(v1, t5, su_h)
            w1 = phd.tile([P, HALF], f32, tag="w1")
            nc.vector.tensor_mul(w1, LUp, su_l)
            tsum = phd.tile([P, HALF], f32, tag="tsum")
            nc.gpsimd.tensor_add(tsum, v1, w1)
            out_t = outp.tile([P, HALF], f32, tag="out_t")
            nc.gpsimd.tensor_add(out_t, tsum, sbc)

            nc.sync.dma_start(out_d[rs, :], out_t)

    nc.compile()
    return nc


_NC_CACHE = {}


def _get_nc(b_loc, mm_dt):
    key = (b_loc, str(mm_dt))
    if key not in _NC_CACHE:
        _NC_CACHE[key] = build_nc(b_loc, mm_dt)
    return _NC_CACHE[key]


def kernel(**inputs):
    mm_dt = f32r if os.environ.get("KERNEL_MM_DT", "f32r") == "f32r" else f32
    b = inputs["hl"].shape[0]
    n_cores = N_CORES
    b_loc = b // n_cores
    nc = _get_nc(b_loc, mm_dt)

    sharded = {"hl", "hr", "xw", "xh"}
    in_maps = []
    for i in range(n_cores):
        m = {}
        for k, v in inputs.items():
            v = np.ascontiguousarray(np.asarray(v, dtype=np.float32))
            if k in sharded:
                m[k] = v[i * b_loc:(i + 1) * b_loc]
            else:
                m[k] = v
        in_maps.append(m)

    res = run_bass_kernel_spmd(nc, in_maps, core_ids=list(range(n_cores)))
    return np.concatenate([r["out"] for r in res.results], axis=0)


# revision 16
# speedup vs baseline: 1.1936x; 1.1936x over previous
"""Trainium2 Bass kernel for nn_ChildHAggregation (gnn_message_passing).

Computation per sample (see docstring math below):
  x = [hl, hr]                                        (1024)
  q_t = (h_t @ qU + qU_b) * su_q + sb_q   t in {l,r}  su/sb from xh
  k_t = (h_t @ kU + kU_b) * su_k + sb_k
  2x2 softmax attention over the two tokens -> per-sample probs p_ij
  x += scores @ [hl, hr]; layernorm(ddof=1) with alpha/beta
  out = (x @ hU + hU_b) * (xh @ hWu + hWu_b) + (xh @ hWb + hWb_b)
      + (xw @ lU + lU_b) * (xh @ lWu + lWu_b) + (xh @ lWb + lWb_b)

Strategy (pure data-parallel over 8 cores, batch 32768 -> 4096/core):
  - batch-major tiles [128 samples x features]; feature-contraction
    matmuls use PE-transposed input tiles as the stationary operand so
    outputs land batch-major in PSUM.
  - 2-token softmax == sigmoid of score differences -> per-sample
    scalars; attention + layernorm are folded into per-sample scalar
    algebra applied AFTER the matmuls:
      x @ (alpha*hU) = a0*M1 + b0*M2 + a1*M3 + b1*M4  (M* = h_t @ hU'half)
      norm fixup: inv * (hu_x - mean * colsum(alpha*hU)), beta@hU folded
      into a bias row.
  - all "U-bias * hyper" terms folded into precomputed weights:
      qWb' = qWb + qWu*diag(qU_b) etc., so q = (h@qU)*su + (xh@qWb'+qb').
    hidden/leaf additive paths combined: WC = hWb' + lWb' (one matmul).
"""

import os
from contextlib import ExitStack

import numpy as np

import concourse.bacc as bacc
import concourse.bass as bass
import concourse.mybir as mybir
import concourse.tile as tile
from concourse.bass_utils import run_bass_kernel_spmd
from concourse.masks import make_identity

N_CORES = 8
B_FULL = 32768
HALF = 512
DIM = 1024
P = 128
EPS = 1e-6
INV_SQRT_HALF = 1.0 / float(np.sqrt(np.float32(HALF)))

f32 = mybir.dt.float32
f32r = mybir.dt.float32r

AX = mybir.AxisListType
ALU = mybir.AluOpType
ACTF = mybir.ActivationFunctionType

W512 = ["qU", "kU", "qWu", "qWb", "kWu", "kWb", "hWu", "hWb", "lWu", "lWb"]


def _mm(ap, mm_dt):
    """Bitcast an fp32 AP to the matmul dtype (f32 or f32r; same bits)."""
    if mm_dt == f32:
        return ap
    return ap.bitcast(mm_dt)


def build_nc(b_loc, mm_dt=f32r, stage=99):
    """Build the per-core Bass program for a local batch of b_loc rows.

    stage truncates the program for hang-bisection:
      1=setup only, 2=+transposes, 3=+phaseA mm/evict, 4=+qk/dots/probs,
      5=+stats algebra, 99=full
    """
    n_tiles = b_loc // P
    assert n_tiles * P == b_loc

    nc = bacc.Bacc("TRN2", target_bir_lowering=False, debug=False,
                   num_devices=1)

    # ---- DRAM I/O (names match setup_inputs) ----
    d = {}
    d["hl"] = nc.dram_tensor("hl", [b_loc, HALF], f32, kind="ExternalInput").ap()
    d["hr"] = nc.dram_tensor("hr", [b_loc, HALF], f32, kind="ExternalInput").ap()
    d["xw"] = nc.dram_tensor("xw", [b_loc, DIM], f32, kind="ExternalInput").ap()
    d["xh"] = nc.dram_tensor("xh", [b_loc, HALF], f32, kind="ExternalInput").ap()
    for w in W512:
        d[w + "_w"] = nc.dram_tensor(w + "_w", [HALF, HALF], f32,
                                     kind="ExternalInput").ap()
        d[w + "_b"] = nc.dram_tensor(w + "_b", [HALF], f32,
                                     kind="ExternalInput").ap()
    for w in ["hU", "lU"]:
        d[w + "_w"] = nc.dram_tensor(w + "_w", [DIM, HALF], f32,
                                     kind="ExternalInput").ap()
        d[w + "_b"] = nc.dram_tensor(w + "_b", [HALF], f32,
                                     kind="ExternalInput").ap()
    d["alpha"] = nc.dram_tensor("alpha", [DIM], f32, kind="ExternalInput").ap()
    d["beta"] = nc.dram_tensor("beta", [DIM], f32, kind="ExternalInput").ap()
    out_d = nc.dram_tensor("out", [b_loc, HALF], f32, kind="ExternalOutput").ap()

    with tile.TileContext(nc) as tc, ExitStack() as ctx:
        # ================= persistent pools =================
        wts = ctx.enter_context(tc.tile_pool(name="wts", bufs=1))
        biasp = ctx.enter_context(tc.tile_pool(name="biasp", bufs=1))

        # final weight tiles (written exactly once, in the matmul dtype, so
        # the fp32r BIR-verifier sees rounded producers)
        wsb = {}
        for w in ["qU", "kU", "qWu", "kWu", "hWu", "lWu", "qWb", "hWb"]:
            wsb[w] = wts.tile([P, 4, HALF], f32, name=f"w_{w}")
        for w in ["hU", "lU"]:
            wsb[w] = wts.tile([P, 8, HALF], f32, name=f"w_{w}")
        alpha_sb = wts.tile([P, 8], f32)
        nc.sync.dma_start(alpha_sb, d["alpha"].rearrange("(c p) -> p c", p=P))
        beta_sb = wts.tile([P, 8], f32)
        nc.sync.dma_start(beta_sb, d["beta"].rearrange("(c p) -> p c", p=P))
        ident = wts.tile([P, P], f32)
        make_identity(nc, ident)

        # persistent broadcast [P, 512] bias tiles
        bc = {}
        for nm in ["qWu_b", "kWu_b", "hWu_b", "lWu_b", "qb", "cb", "cs"]:
            bc[nm] = biasp.tile([P, HALF], f32, name=f"bc_{nm}")

        # ---------------- one-time setup ----------------
        with tc.tile_pool(name="setup", bufs=1) as sp, \
                tc.tile_pool(name="psum_setup", bufs=2, space="PSUM") as psum_setup:

            def bias_row(nm):
                r = sp.tile([1, HALF], f32, tag="row", bufs=2, name=f"row_{nm}")
                nc.sync.dma_start(r, d[nm][None, :])
                return r

            def bcast(dst, row_ap):
                nc.gpsimd.partition_broadcast(dst, row_ap)

            def tmp_bc(nm, row_ap):
                t = sp.tile([P, HALF], f32, tag="tbc", bufs=5, name=f"tbc_{nm}")
                bcast(t, row_ap)
                return t

            # su-evict biases straight to persistent broadcasts
            for nm in ["qWu_b", "kWu_b", "hWu_b", "lWu_b"]:
                bcast(bc[nm], bias_row(nm))

            # temp broadcasts used by folds / combined rows
            qUb_bc = tmp_bc("qU_b", bias_row("qU_b"))
            lUb_bc = tmp_bc("lU_b", bias_row("lU_b"))

            # combined bias rows (computed on broadcast tiles):
            # qb' = qWb_b + qU_b*qWu_b ; kb' likewise
            ftmp = sp.tile([P, HALF], f32, tag="ftmp")
            bcast(bc["qb"], bias_row("qWb_b"))
            nc.vector.tensor_mul(ftmp, qUb_bc, bc["qWu_b"])
            nc.vector.tensor_add(bc["qb"], bc["qb"], ftmp)

            # weights: DMA into rotating temps, finals written once (rounded)
            def wtemp(w, nch):
                t = sp.tile([P, nch, HALF], f32, tag=f"wtmp{nch}",
                            bufs=(3 if nch == 4 else 1), name=f"wtmp_{w}")
                nc.sync.dma_start(t, d[w + "_w"].rearrange("(c p) o -> p c o", p=P))
                return t

            for w in ["qU", "kU", "qWu", "kWu", "hWu", "lWu"]:
                t = wtemp(w, 4)
                for c in range(4):
                    nc.vector.tensor_copy(_mm(wsb[w][:, c, :], mm_dt), t[:, c, :])

            # hU: bh/cs matmuls on raw hU, then alpha-scale into final
            hU_tmp = wtemp("hU", 8)
            bh_ps = psum_setup.tile([1, HALF], f32)
            cs_ps = psum_setup.tile([1, HALF], f32)
            for c in range(8):
                nc.tensor.matmul(bh_ps, beta_sb[:, c:c + 1], hU_tmp[:, c, :],
                                 start=(c == 0), stop=(c == 7))
            for c in range(8):
                nc.tensor.matmul(cs_ps, alpha_sb[:, c:c + 1], hU_tmp[:, c, :],
                                 start=(c == 0), stop=(c == 7))
            bh_row = sp.tile([1, HALF], f32, tag="row", bufs=2)
            nc.vector.tensor_add(bh_row, bh_ps, bias_row("hU_b"))
            cs_row = sp.tile([1, HALF], f32, tag="row", bufs=2)
            nc.vector.tensor_copy(cs_row, cs_ps)
            bcast(bc["cs"], cs_row)
            bh_bc = tmp_bc("bh", bh_row)
            for c in range(8):
                nc.scalar.activation(_mm(wsb["hU"][:, c, :], mm_dt),
                                     hU_tmp[:, c, :],
                                     ACTF.Copy, scale=alpha_sb[:, c:c + 1])

            lU_tmp = wtemp("lU", 8)
            for c in range(8):
                nc.vector.tensor_copy(_mm(wsb["lU"][:, c, :], mm_dt),
                                      lU_tmp[:, c, :])

            # cb = (hWb_b + bh*hWu_b) + (lWb_b + lU_b*lWu_b)
            bcast(bc["cb"], bias_row("hWb_b"))
            nc.vector.tensor_mul(ftmp, bh_bc, bc["hWu_b"])
            nc.vector.tensor_add(bc["cb"], bc["cb"], ftmp)
            lWbb_bc = tmp_bc("lWb_b", bias_row("lWb_b"))
            nc.vector.tensor_add(bc["cb"], bc["cb"], lWbb_bc)
            nc.vector.tensor_mul(ftmp, lUb_bc, bc["lWu_b"])
            nc.vector.tensor_add(bc["cb"], bc["cb"], ftmp)

            # folded weight matrices:
            # qWb' = qWb + qWu*diag(qU_b) ; kWb' = kWb + kWu*diag(kU_b)
            # WC   = hWb + hWu*diag(bh) + lWb + lWu*diag(lU_b)
            qWb_tmp = wtemp("qWb", 4)
            for c in range(4):
                nc.vector.tensor_mul(ftmp, wsb["qWu"][:, c, :], qUb_bc)
                nc.vector.tensor_add(_mm(wsb["qWb"][:, c, :], mm_dt),
                                     qWb_tmp[:, c, :], ftmp)
            hWb_tmp = wtemp("hWb", 4)
            lWb_tmp = wtemp("lWb", 4)
            for c in range(4):
                nc.vector.tensor_mul(ftmp, wsb["hWu"][:, c, :], bh_bc)
                nc.vector.tensor_add(hWb_tmp[:, c, :], hWb_tmp[:, c, :], ftmp)
                nc.vector.tensor_add(hWb_tmp[:, c, :], hWb_tmp[:, c, :],
                                     lWb_tmp[:, c, :])
                nc.vector.tensor_mul(ftmp, wsb["lWu"][:, c, :], lUb_bc)
                nc.vector.tensor_add(_mm(wsb["hWb"][:, c, :], mm_dt),
                                     hWb_tmp[:, c, :], ftmp)

        # ================= main loop pools =================
        inp = ctx.enter_context(tc.tile_pool(name="inp", bufs=2))
        tsp = ctx.enter_context(tc.tile_pool(name="tsp", bufs=2))
        pha = ctx.enter_context(tc.tile_pool(name="pha", bufs=1))
        scr = ctx.enter_context(tc.tile_pool(name="scr", bufs=4))
        tinyp = ctx.enter_context(tc.tile_pool(name="tinyp", bufs=2))
        phd = ctx.enter_context(tc.tile_pool(name="phd", bufs=1))
        outp = ctx.enter_context(tc.tile_pool(name="outp", bufs=2))
        tp_ps = ctx.enter_context(tc.tile_pool(name="tp_ps", bufs=2, space="PSUM"))
        mm_ps = ctx.enter_context(tc.tile_pool(name="mm_ps", bufs=6, space="PSUM"))

        for i in range(n_tiles):
            rs = bass.ts(i, P)
            # ---- loads ----
            hl_t = inp.tile([P, HALF], f32, tag="hl")
            nc.sync.dma_start(hl_t, d["hl"][rs, :])
            hr_t = inp.tile([P, HALF], f32, tag="hr")
            nc.sync.dma_start(hr_t, d["hr"][rs, :])
            xh_t = inp.tile([P, HALF], f32, tag="xh")
            nc.sync.dma_start(xh_t, d["xh"][rs, :])
            xw_t = inp.tile([P, DIM], f32, tag="xw")
            nc.sync.dma_start(xw_t, d["xw"][rs, :])


            def stage_out(src_ap):
                ot = outp.tile([P, HALF], f32, tag="out_t", name=f"out_stage_{i}")
                nc.vector.tensor_copy(ot, src_ap)
                nc.sync.dma_start(out_d[rs, :], ot)

            if stage == 1:
                stage_out(hl_t)
                continue

            # ---- PE transposes (feature-major stationaries) ----
            def transpose_to(src, ncols, tg):
                sb = tsp.tile([P, ncols * P], f32, tag=tg, name=f"T_{tg}_{i}")
                for g in range(0, ncols, 4):
                    ps = tp_ps.tile([P, 4 * P], f32, tag="tp",
                                    name=f"tps_{tg}_{g}_{i}")
                    gw = min(4, ncols - g)
                    for c in range(gw):
                        nc.tensor.transpose(
                            ps[:, c * P:(c + 1) * P],
                            src[:, (g + c) * P:(g + c + 1) * P],
                            ident)
                    # evict writes the matmul dtype (rounds for fp32r)
                    nc.scalar.copy(_mm(sb[:, g * P:(g + gw) * P], mm_dt),
                                   ps[:, :gw * P])
                return sb

            hlT = transpose_to(hl_t, 4, "ThL")
            hrT = transpose_to(hr_t, 4, "ThR")
            xhT = transpose_to(xh_t, 4, "TxH")
            xwT = transpose_to(xw_t, 8, "TxW")

            if stage == 2:
                stage_out(hlT)
                continue

            # ---- row stats of hl / hr (for fused layernorm algebra) ----
            skip_stats = (stage == 22)
            smask = int(os.environ.get("KERNEL_STATS_MASK", "7"))
            if not skip_stats:
                sl = tinyp.tile([P, 1], f32, tag="sl")
                sr = tinyp.tile([P, 1], f32, tag="sr")
                ql = tinyp.tile([P, 1], f32, tag="ql")
                qr = tinyp.tile([P, 1], f32, tag="qr")
                cr2 = tinyp.tile([P, 1], f32, tag="cr2")
                if smask & 1:
                    nc.vector.reduce_sum(sl, hl_t, axis=AX.X)
                    nc.vector.reduce_sum(sr, hr_t, axis=AX.X)
                if smask & 2:
                    s1 = scr.tile([P, HALF], f32, tag="scr", name=f"scr_ql_{i}")
                    nc.scalar.activation(s1, hl_t, ACTF.Square, accum_out=ql)
                    s2 = scr.tile([P, HALF], f32, tag="scr", name=f"scr_qr_{i}")
                    nc.scalar.activation(s2, hr_t, ACTF.Square, accum_out=qr)
                if smask & 4:
                    s3 = scr.tile([P, HALF], f32, tag="scr", name=f"scr_cr_{i}")
                    nc.vector.scalar_tensor_tensor(
                        s3, hl_t, 0.0, hr_t, ALU.bypass, ALU.mult,
                        accum_out=cr2)
            if stage == 21:
                stage_out(hlT)
                continue

            # ---- phase A matmuls ----
            def unit(tag):
                return mm_ps.tile([P, HALF], f32, tag="mm", name=f"ps_{tag}_{i}")

            SUq, SBq, TU = unit("SUq"), unit("SBq"), unit("TU")
            for c in range(4):
                lhs = _mm(xhT[:, bass.ts(c, P)], mm_dt)
                st, sp_ = (c == 0), (c == 3)
                nc.tensor.matmul(SUq, lhs, _mm(wsb["qWu"][:, c, :], mm_dt), start=st, stop=sp_)
                nc.tensor.matmul(SBq, lhs, _mm(wsb["qWb"][:, c, :], mm_dt), start=st, stop=sp_)
                nc.tensor.matmul(TU, lhs, _mm(wsb["kWu"][:, c, :], mm_dt), start=st, stop=sp_)
            A_l, C_l = unit("A_l"), unit("C_l")
            for c in range(4):
                lhs = _mm(hlT[:, bass.ts(c, P)], mm_dt)
                st, sp_ = (c == 0), (c == 3)
                nc.tensor.matmul(A_l, lhs, _mm(wsb["qU"][:, c, :], mm_dt), start=st, stop=sp_)
                nc.tensor.matmul(C_l, lhs, _mm(wsb["kU"][:, c, :], mm_dt), start=st, stop=sp_)
            A_r, C_r = unit("A_r"), unit("C_r")
            for c in range(4):
                lhs = _mm(hrT[:, bass.ts(c, P)], mm_dt)
                st, sp_ = (c == 0), (c == 3)
                nc.tensor.matmul(A_r, lhs, _mm(wsb["qU"][:, c, :], mm_dt), start=st, stop=sp_)
                nc.tensor.matmul(C_r, lhs, _mm(wsb["kU"][:, c, :], mm_dt), start=st, stop=sp_)

            # ---- phase A elementwise (score-difference trick) ----
            # d0 = q_l . (k_l - k_r), d1 = q_r . (k_l - k_r);
            # k_l - k_r = (C_l - C_r) * tu  (kU_b and the k additive
            # hyper term cancel in the difference), and with the qWb'
            # fold q_t = A_t * su + sbq, so
            # d_t = sum(A_t * su * dk) + sum(sbq * dk).
            su = pha.tile([P, HALF], f32, tag="su")
            nc.vector.tensor_add(su, SUq, bc["qWu_b"])
            sbq = pha.tile([P, HALF], f32, tag="sbq")
            nc.vector.tensor_add(sbq, SBq, bc["qb"])
            tu = pha.tile([P, HALF], f32, tag="tu")
            nc.vector.tensor_add(tu, TU, bc["kWu_b"])

            if stage == 3:
                stage_out(su)
                continue

            crsb = pha.tile([P, HALF], f32, tag="crsb")
            nc.scalar.copy(crsb, C_r)
            dk = pha.tile([P, HALF], f32, tag="dk")
            nc.vector.tensor_sub(dk, C_l, crsb)
            nc.vector.tensor_mul(dk, dk, tu)
            u = pha.tile([P, HALF], f32, tag="u")
            nc.vector.tensor_mul(u, su, dk)

            stats = tinyp.tile([P, 4], f32, tag="stats")
            for j, (aa, bb) in enumerate([(sbq, dk), (A_l, u), (A_r, u)]):
                sd = scr.tile([P, HALF], f32, tag="scr", name=f"scr_dot{j}_{i}")
                nc.vector.scalar_tensor_tensor(
                    sd, aa, 0.0, bb, ALU.bypass, ALU.mult,
                    accum_out=stats[:, j:j + 1])

            # ---- 2-way softmax via sigmoid ----
            diffs = tinyp.tile([P, 2], f32, tag="diffs")
            nc.vector.tensor_add(diffs, stats[:, 1:3],
                                 stats[:, 0:1].broadcast_to([P, 2]))
            probs = tinyp.tile([P, 2], f32, tag="probs")
            nc.scalar.activation(probs, diffs, ACTF.Sigmoid, scale=INV_SQRT_HALF)
            a0 = tinyp.tile([P, 1], f32, tag="a0")
            nc.scalar.activation(a0, probs[:, 0:1], ACTF.Copy, bias=1.0)
            b0 = tinyp.tile([P, 1], f32, tag="b0")
            nc.scalar.activation(b0, probs[:, 0:1], ACTF.Copy, scale=-1.0, bias=1.0)
            a1 = probs[:, 1:2]
            b1 = tinyp.tile([P, 1], f32, tag="b1")
            nc.scalar.activation(b1, probs[:, 1:2], ACTF.Copy, scale=-1.0, bias=2.0)

            if stage == 4:
                stage_out(u)
                continue

            # ---- layernorm stats from folded algebra ----
            e0 = tinyp.tile([P, 1], f32, tag="e0")
            nc.vector.tensor_add(e0, a0, a1)
            e1 = tinyp.tile([P, 1], f32, tag="e1")
            nc.vector.tensor_add(e1, b0, b1)
            sumx = tinyp.tile([P, 1], f32, tag="sumx")
            nc.vector.tensor_mul(sumx, sl, e0)
            nc.vector.scalar_tensor_tensor(sumx, sr, e1, sumx, ALU.mult, ALU.add)
            f0 = tinyp.tile([P, 1], f32, tag="f0")
            nc.vector.tensor_mul(f0, a0, a0)
            nc.vector.scalar_tensor_tensor(f0, a1, a1, f0, ALU.mult, ALU.add)
            f1 = tinyp.tile([P, 1], f32, tag="f1")
            nc.vector.tensor_mul(f1, b0, b0)
            nc.vector.scalar_tensor_tensor(f1, b1, b1, f1, ALU.mult, ALU.add)
            f2 = tinyp.tile([P, 1], f32, tag="f2")
            nc.vector.tensor_mul(f2, a0, b0)
            nc.vector.scalar_tensor_tensor(f2, a1, b1, f2, ALU.mult, ALU.add)
            nc.scalar.activation(f2, f2, ACTF.Copy, scale=2.0)
            ssq = tinyp.tile([P, 1], f32, tag="ssq")
            nc.vector.tensor_mul(ssq, ql, f0)
            nc.vector.scalar_tensor_tensor(ssq, qr, f1, ssq, ALU.mult, ALU.add)
            nc.vector.scalar_tensor_tensor(ssq, cr2, f2, ssq, ALU.mult, ALU.add)
            mean = tinyp.tile([P, 1], f32, tag="mean")
            nc.scalar.activation(mean, sumx, ACTF.Copy, scale=1.0 / DIM)
            m2x = tinyp.tile([P, 1], f32, tag="m2x")
            nc.vector.tensor_mul(m2x, sumx, sumx)
            varn = tinyp.tile([P, 1], f32, tag="varn")
            nc.vector.scalar_tensor_tensor(varn, m2x, -1.0 / DIM, ssq,
                                           ALU.mult, ALU.add)
            stde = tinyp.tile([P, 1], f32, tag="stde")
            nc.scalar.activation(stde, varn, ACTF.Sqrt, scale=1.0 / (DIM - 1))
            nc.scalar.activation(stde, stde, ACTF.Copy, bias=EPS)
            rinv = tinyp.tile([P, 1], f32, tag="rinv")
            nc.vector.reciprocal(rinv, stde)
            nrinv = tinyp.tile([P, 1], f32, tag="nrinv")
            nc.scalar.activation(nrinv, rinv, ACTF.Copy, scale=-1.0)

            if stage == 5:
                stage_out(dk)
                continue

            # ---- phase D matmuls ----
            M1, M3 = unit("M1"), unit("M3")
            for c in range(4):
                lhs = _mm(hlT[:, bass.ts(c, P)], mm_dt)
                st, sp_ = (c == 0), (c == 3)
                nc.tensor.matmul(M1, lhs, _mm(wsb["hU"][:, c, :], mm_dt), start=st, stop=sp_)
                nc.tensor.matmul(M3, lhs, _mm(wsb["hU"][:, 4 + c, :], mm_dt), start=st, stop=sp_)
            M2, M4 = unit("M2"), unit("M4")
            for c in range(4):
                lhs = _mm(hrT[:, bass.ts(c, P)], mm_dt)
                st, sp_ = (c == 0), (c == 3)
                nc.tensor.matmul(M2, lhs, _mm(wsb["hU"][:, c, :], mm_dt), start=st, stop=sp_)
                nc.tensor.matmul(M4, lhs, _mm(wsb["hU"][:, 4 + c, :], mm_dt), start=st, stop=sp_)
            HSU, LSU, SBC = unit("HSU"), unit("LSU"), unit("SBC")
            for c in range(4):
                lhs = _mm(xhT[:, bass.ts(c, P)], mm_dt)
                st, sp_ = (c == 0), (c == 3)
                nc.tensor.matmul(HSU, lhs, _mm(wsb["hWu"][:, c, :], mm_dt), start=st, stop=sp_)
                nc.tensor.matmul(LSU, lhs, _mm(wsb["lWu"][:, c, :], mm_dt), start=st, stop=sp_)
                nc.tensor.matmul(SBC, lhs, _mm(wsb["hWb"][:, c, :], mm_dt), start=st, stop=sp_)
            LUp = unit("LU")
            for c in range(8):
                nc.tensor.matmul(LUp, _mm(xwT[:, bass.ts(c, P)], mm_dt),
                                 _mm(wsb["lU"][:, c, :], mm_dt),
                                 start=(c == 0), stop=(c == 7))

            # ---- hidden path: hu_x = a0*M1 + b0*M2 + a1*M3 + b1*M4 ----
            t_hu = phd.tile([P, HALF], f32, tag="t_hu")
            nc.scalar.activation(t_hu, M1, ACTF.Copy, scale=a0)
            nc.vector.scalar_tensor_tensor(t_hu, M2, b0, t_hu, ALU.mult, ALU.add)
            nc.vector.scalar_tensor_tensor(t_hu, M3, a1, t_hu, ALU.mult, ALU.add)
            nc.vector.scalar_tensor_tensor(t_hu, M4, b1, t_hu, ALU.mult, ALU.add)
            # t5 = cs*mean - hu_x ; u1 = -inv * t5 = inv*(hu_x - cs*mean)
            t5 = phd.tile([P, HALF], f32, tag="t5")
            nc.vector.scalar_tensor_tensor(t5, bc["cs"], mean, t_hu,
                                           ALU.mult, ALU.subtract)
            nc.scalar.activation(t5, t5, ACTF.Copy, scale=nrinv)

            su_h = phd.tile([P, HALF], f32, tag="su_h")
            nc.vector.tensor_add(su_h, HSU, bc["hWu_b"])
            su_l = phd.tile([P, HALF], f32, tag="su_l")
            nc.vector.tensor_add(su_l, LSU, bc["lWu_b"])
            sbc = phd.tile([P, HALF], f32, tag="sbc")
            nc.vector.tensor_add(sbc, SBC, bc["cb"])

            v1 = phd.tile([P, HALF], f32, tag="v1")
            nc.vector.tensor_mul(v1, t5, su_h)
            w1 = phd.tile([P, HALF], f32, tag="w1")
            nc.vector.tensor_mul(w1, LUp, su_l)
            tsum = phd.tile([P, HALF], f32, tag="tsum")
            nc.gpsimd.tensor_add(tsum, v1, sbc)
            out_t = outp.tile([P, HALF], f32, tag="out_t")
            nc.gpsimd.tensor_add(out_t, tsum, w1)

            nc.sync.dma_start(out_d[rs, :], out_t)

    nc.compile()
    return nc


_NC_CACHE = {}


def _get_nc(b_loc, mm_dt):
    key = (b_loc, str(mm_dt))
    if key not in _NC_CACHE:
        _NC_CACHE[key] = build_nc(b_loc, mm_dt)
    return _NC_CACHE[key]


def kernel(**inputs):
    mm_dt = f32r if os.environ.get("KERNEL_MM_DT", "f32r") == "f32r" else f32
    b = inputs["hl"].shape[0]
    n_cores = N_CORES
    b_loc = b // n_cores
    nc = _get_nc(b_loc, mm_dt)

    sharded = {"hl", "hr", "xw", "xh"}
    in_maps = []
    for i in range(n_cores):
        m = {}
        for k, v in inputs.items():
            v = np.ascontiguousarray(np.asarray(v, dtype=np.float32))
            if k in sharded:
                m[k] = v[i * b_loc:(i + 1) * b_loc]
            else:
                m[k] = v
        in_maps.append(m)

    res = run_bass_kernel_spmd(nc, in_maps, core_ids=list(range(n_cores)))
    return np.concatenate([r["out"] for r in res.results], axis=0)
